# revision 1
# baseline (speedup 1.0000x reference)
"""Trainium2 Bass kernel for nn_ClassifyModel_70970039599212 (3-layer GraphConv +
global attention pooling + MLP classifier) distributed over 8 NeuronCores.

Strategy (dst-partitioned graph parallelism):
  - Nodes are permuted and packed into 392 windows of 128 (balanced by
    in-degree so every window has a near-equal edge count); each of the 8
    cores owns 49 consecutive windows (6272 nodes).
  - Each core owns the edges whose dst falls in its windows (~E/8). For each
    window, edge tiles of 128 are gathered from the (replicated) feature
    table via indirect DMA, then reduced into the window's 128 node rows by a
    TensorEngine matmul against a data-built selector matrix
    S[e, n] = w_e * (dst_rel[e] == n), w_e = out_deg(src)^-1/2 * in_deg(dst)^-1/2,
    which applies both GraphConv norms inline.
  - The aggregated window is transposed (PE) and multiplied by the layer
    weight; ReLU+bias applied; the slice is AllGathered so the next layer can
    gather from the full table. Layer 2 output feeds pooling directly
    (no AllGather): gate -> exp -> weighted one-hot-graph matmuls accumulate
    per-graph sums in PSUM; a single [257, 64] AllReduce combines cores; the
    tiny MLP runs replicated on every core.
"""
import os
import sys
import types

import numpy as np
import orjson

import concourse.bass as bass
import concourse.mybir as mybir
import concourse.tile as tile
import concourse.bass_utils as bass_utils
import concourse.bass2jax as bass2jax
from concourse.bass_utils import run_bass_kernel_spmd
from bass_rust import ScopedClock, SyncInfo

# ---------------------------------------------------------------------------
# Compat patches for this walrus build: it rejects instructions carrying more
# than one semaphore wait (two for EventSemaphore). Split offenders.
# ---------------------------------------------------------------------------
_WAIT_CAP = {"EventSemaphore": 2}


def _patched_drain_and_barrier(self, tick_clock, wait_clock):
    nc = self.nc
    drain_inst = nc.sync.drain()
    wait_clock.add_sem_waits(
        drain_inst.ins, ScopedClock({None: tick_clock.global_clock})
    )
    si = drain_inst.ins.sync_info
    waits = list(si.on_wait)
    if len(waits) > 1:
        drain_inst.ins.sync_info = SyncInfo(
            on_wait=[waits[0]], on_update=list(si.on_update)
        )
        for w in waits[1:]:
            extra = nc.sync.drain()
            extra.ins.sync_info = SyncInfo(on_wait=[w], on_update=[])
    nc.all_engine_barrier()
    assert self.sems is not None
    popped = nc._tile_sem_poison_stack.pop()
    assert popped is self._sem_poison
    nc.clear_and_free_semaphores(list(self.sems.allocated().values()))
    nc.all_engine_barrier()


def _split_multiwait_bir(bir_json: bytes) -> bytes:
    m = orjson.loads(bir_json)
    counter = 0
    changed = False
    for fn in m["functions"]:
        for bb in fn["blocks"]:
            out = []
            for ins in bb["instructions"]:
                si = ins.get("sync_info")
                if si:
                    waits = si.get("on_wait") or []
                    cap = _WAIT_CAP.get(ins.get("opcode"), 1)
                    if len(waits) > cap:
                        changed = True
                        extra = waits[:-cap]
                        si["on_wait"] = waits[-cap:]
                        for i in range(0, len(extra), 2):
                            counter += 1
                            out.append(
                                {
                                    "debug": ins.get("debug", 0),
                                    "engine": ins["engine"],
                                    "ins": [],
                                    "name": f"I-wsplit-{counter}",
                                    "opcode": "EventSemaphore",
                                    "outs": [],
                                    "sync_info": {
                                        "on_update": [],
                                        "on_wait": extra[i : i + 2],
                                    },
                                }
                            )
                out.append(ins)
            bb["instructions"] = out
    return orjson.dumps(m) if changed else bir_json


_orig_compile_bir_kernel = bass_utils.compile_bir_kernel


def _patched_compile_bir_kernel(bir_json, tmpdir, neff_name="file.neff"):
    if isinstance(bir_json, str):
        bir_json = bir_json.encode()
    return _orig_compile_bir_kernel(
        _split_multiwait_bir(bir_json), tmpdir, neff_name
    )


_PATCHED = False


def _install_patches():
    global _PATCHED
    if _PATCHED:
        return
    tile.TileContext._drain_and_barrier = _patched_drain_and_barrier
    bass_utils.compile_bir_kernel = _patched_compile_bir_kernel
    bass2jax.compile_bir_kernel = _patched_compile_bir_kernel
    _PATCHED = True


# ---------------------------------------------------------------------------
# Problem constants (hardcoded per contract)
# ---------------------------------------------------------------------------
N, E, B = 50000, 600000, 64
IN_DIM, HID, OUT_DIM = 128, 256, 256
BN_EPS = 1e-5
P = 128
NCORES = 8
W_TOTAL = 392            # node windows of 128 -> 50176 padded nodes
NPAD = W_TOTAL * P
W_CORE = W_TOTAL // NCORES      # 49 windows per core
NODES_CORE = W_CORE * P         # 6272
AG_CHUNKS = 7                   # pipelined AllGather chunks per layer
AG_CW = NODES_CORE // AG_CHUNKS  # 896 rows per core per chunk

F32 = mybir.dt.float32
BF16 = mybir.dt.bfloat16
I32 = mybir.dt.int32
AX = mybir.AxisListType
OP = mybir.AluOpType
ACT = mybir.ActivationFunctionType


# ---------------------------------------------------------------------------
# Host-side preprocessing
# ---------------------------------------------------------------------------
def _preprocess(x, src, dst, graph_ids):
    src = np.asarray(src, np.int64)
    dst = np.asarray(dst, np.int64)
    out_deg = np.bincount(src, minlength=N).astype(np.float32)
    in_deg = np.bincount(dst, minlength=N).astype(np.float32)
    norm_src = np.maximum(out_deg, 1.0) ** -0.5
    norm_dst = np.maximum(in_deg, 1.0) ** -0.5

    # Pack nodes into W_TOTAL windows of P, balancing per-window edge count:
    # sort (padded) nodes by in-degree desc, snake-assign across windows.
    deg_all = np.zeros(NPAD, np.int64)
    deg_all[:N] = in_deg.astype(np.int64)
    order = np.argsort(-deg_all, kind="stable")
    win_of = np.empty(NPAD, np.int64)
    slot_of = np.empty(NPAD, np.int64)
    fwd = np.arange(W_TOTAL)
    rev = fwd[::-1]
    for r in range(P):
        seg = order[r * W_TOTAL : (r + 1) * W_TOTAL]
        ws = fwd if (r % 2 == 0) else rev
        win_of[seg] = ws
        slot_of[seg] = r
    perm = win_of * P + slot_of       # old (padded) id -> new id

    new_src = perm[src]
    new_dst = perm[dst]
    w_edge = (norm_src[src] * norm_dst[dst]).astype(np.float32)
    win_e = new_dst // P
    rel_e = (new_dst % P).astype(np.float32)

    cnt = np.bincount(win_e, minlength=W_TOTAL)
    T_w = int(np.ceil(cnt.max() / P))
    SLOTS = T_w * P
    TC = W_CORE * T_w

    # order edges within each window by the AG chunk of their source, so a
    # tile's table dependency is a prefix of the chunk-major table
    chunk_e = (new_src % NODES_CORE) // AG_CW
    eorder = np.lexsort((chunk_e, win_e))
    starts = np.zeros(W_TOTAL + 1, np.int64)
    starts[1:] = np.cumsum(cnt)
    rank = np.arange(E) - starts[win_e[eorder]]

    idx_arr = np.zeros((W_TOTAL, SLOTS), np.int32)
    rel_arr = np.full((W_TOTAL, SLOTS), 999.0, np.float32)  # pad -> no match
    we = win_e[eorder]
    idx_arr[we, rank] = new_src[eorder].astype(np.int32)
    rel_arr[we, rank] = rel_e[eorder]

    # chunk-major table row for h1/h2: node n -> (s*NCORES + r)*AG_CW + j
    r_ = idx_arr.astype(np.int64) // NODES_CORE
    off_ = idx_arr.astype(np.int64) % NODES_CORE
    s_ = off_ // AG_CW
    j_ = off_ % AG_CW
    idx2_arr = ((s_ * NCORES + r_) * AG_CW + j_).astype(np.int32)
    # per-tile-position chunk extents (max over all windows -> SPMD-identical)
    ext = s_.reshape(W_TOTAL, T_w, P).max(axis=(0, 2))  # [T_w]
    ext = np.maximum.accumulate(ext).astype(np.int64)

    # lane-major per-core views [128, TC]
    def lane_major(a):
        # [W_TOTAL, T_w, P] -> per core [49*T_w, P].T
        a3 = a.reshape(W_TOTAL, T_w, P)
        return [
            np.ascontiguousarray(
                a3[c * W_CORE : (c + 1) * W_CORE].reshape(TC, P).T
            )
            for c in range(NCORES)
        ]

    idx_c = lane_major(idx_arr)
    idx2_c = lane_major(idx2_arr)
    rel_c = lane_major(rel_arr)

    import ml_dtypes
    # x~ = x * out_deg^-1/2, stored bf16 (aggregation input table)
    x_perm = np.zeros((NPAD, IN_DIM), np.float32)
    x_perm[perm[:N]] = np.asarray(x, np.float32) * norm_src[:, None]
    x_perm = x_perm.astype(ml_dtypes.bfloat16)

    # per-new-node norm vectors, lane-major [128, W_CORE] per core
    ns_all = np.ones(NPAD, np.float32)
    ns_all[perm[:N]] = norm_src
    nd_all = np.ones(NPAD, np.float32)
    nd_all[perm[:N]] = norm_dst

    def lane_major_node(v):
        v2 = v.reshape(W_TOTAL, P)
        return [
            np.ascontiguousarray(v2[c * W_CORE : (c + 1) * W_CORE].T)
            for c in range(NCORES)
        ]

    ns_c = lane_major_node(ns_all)
    nd_c = lane_major_node(nd_all)


    gid_all = np.full(NPAD, 1.0e9, np.float32)
    gid_all[perm[:N]] = np.asarray(graph_ids, np.float32)
    gid_c = lane_major_node(gid_all)
    return dict(
        T_w=T_w, TC=TC, idx_c=idx_c, idx2_c=idx2_c, ext=ext, rel_c=rel_c,
        x_perm=x_perm, gid_c=gid_c, ns_c=ns_c, nd_c=nd_c,
    )


# ---------------------------------------------------------------------------
# Device program
# ---------------------------------------------------------------------------
def _build_nc(T_w, gate_b_val, ext=None, dds=65536):
    if ext is None:
        ext = [AG_CHUNKS - 1] * T_w
    _install_patches()
    TC = W_CORE * T_w
    nc = bass.Bass(dynamic_dma_scratch_size=dds)

    # I/O
    xin = nc.declare_dram_parameter("xin", [NPAD, IN_DIM], BF16, isOutput=False)
    idxs_d = nc.declare_dram_parameter("idxs", [P, TC], I32, isOutput=False)
    idxs2_d = nc.declare_dram_parameter("idxs2", [P, TC], I32, isOutput=False)
    ns_d = nc.declare_dram_parameter("nsrc", [P, W_CORE], F32, isOutput=False)
    nd_d = nc.declare_dram_parameter("ndst", [P, W_CORE], F32, isOutput=False)
    dstrel_d = nc.declare_dram_parameter("dstrel", [P, TC], F32, isOutput=False)
    gid_d = nc.declare_dram_parameter("gid", [P, W_CORE], F32, isOutput=False)
    iota_d = nc.declare_dram_parameter("iota", [P, P], F32, isOutput=False)
    eye_d = nc.declare_dram_parameter("eye", [P, P], F32, isOutput=False)
    ones_d = nc.declare_dram_parameter("ones1", [1, P], F32, isOutput=False)
    W0_d = nc.declare_dram_parameter("W0", [IN_DIM, HID], F32, isOutput=False)
    W1_d = nc.declare_dram_parameter("W1", [HID, HID], F32, isOutput=False)
    W2_d = nc.declare_dram_parameter("W2", [HID, OUT_DIM], F32, isOutput=False)
    b0_d = nc.declare_dram_parameter("b0b", [P, HID], F32, isOutput=False)
    b1_d = nc.declare_dram_parameter("b1b", [P, HID], F32, isOutput=False)
    b2_d = nc.declare_dram_parameter("b2b", [P, OUT_DIM], F32, isOutput=False)
    gw_d = nc.declare_dram_parameter("gwb", [P, OUT_DIM], F32, isOutput=False)
    m1w_d = nc.declare_dram_parameter("m1w", [OUT_DIM, 128], F32, isOutput=False)
    m1b_d = nc.declare_dram_parameter("m1b", [128, 1], F32, isOutput=False)
    m2w_d = nc.declare_dram_parameter("m2w", [128, 64], F32, isOutput=False)
    m2b_d = nc.declare_dram_parameter("m2b", [64, 1], F32, isOutput=False)
    m3w_d = nc.declare_dram_parameter("m3w", [64, 2], F32, isOutput=False)
    m3b_d = nc.declare_dram_parameter("m3b", [2, 1], F32, isOutput=False)
    out_d = nc.declare_dram_parameter("out", [2, B], F32, isOutput=True)
    debug = bool(int(os.environ.get("BASS_GNN_DEBUG", "0")))
    if debug:
        dbg1_d = nc.declare_dram_parameter("dbg1", [NODES_CORE, HID], F32, isOutput=True)
        dbg2_d = nc.declare_dram_parameter("dbg2", [NODES_CORE, HID], F32, isOutput=True)
        dbgp_d = nc.declare_dram_parameter("dbgp", [2 * P + 1, B], F32, isOutput=True)

    with tile.TileContext(nc) as tc:
        # the race detector flags disjoint chunked-AllGather writes into one
        # Shared tensor as a multi-writer violation; the chunks are disjoint.
        tc.race_detector_enabled = False
        with (
            tc.tile_pool(name="consts", bufs=1) as cp,
            tc.tile_pool(name="dram", bufs=1, space="DRAM") as dp,
        ):
            # ---- load constants ----
            idxs = cp.tile([P, TC], I32)
            idxs2 = cp.tile([P, TC], I32)
            nsrc = cp.tile([P, W_CORE], F32)
            ndst = cp.tile([P, W_CORE], F32)
            dstrel = cp.tile([P, TC], F32)
            gid = cp.tile([P, W_CORE], F32)
            iota = cp.tile([P, P], F32)
            eye = cp.tile([P, P], F32)
            ones1 = cp.tile([1, P], F32)
            # >128-row weights stored as row-chunks side by side in SBUF
            W0 = cp.tile([P, HID], F32)
            W1 = cp.tile([P, 2 * HID], F32)
            W2 = cp.tile([P, 2 * OUT_DIM], F32)
            b0 = cp.tile([P, HID], F32)
            b1 = cp.tile([P, HID], F32)
            b2 = cp.tile([P, OUT_DIM], F32)
            gw = cp.tile([P, OUT_DIM], F32)
            m1w = cp.tile([P, 2 * 128], F32)
            m1b = cp.tile([128, 1], F32)
            m2w = cp.tile([128, 64], F32)
            m2b = cp.tile([64, 1], F32)
            m3w = cp.tile([64, 2], F32)
            m3b = cp.tile([2, 1], F32)
            for t, d in [
                (idxs, idxs_d), (idxs2, idxs2_d),
                (nsrc, ns_d), (ndst, nd_d), (dstrel, dstrel_d),
                (gid, gid_d), (iota, iota_d), (eye, eye_d), (ones1, ones_d),
                (W0, W0_d),
                (b0, b0_d), (b1, b1_d), (b2, b2_d), (gw, gw_d),
                (m1b, m1b_d), (m2w, m2w_d), (m2b, m2b_d),
                (m3w, m3w_d), (m3b, m3b_d),
            ]:
                nc.sync.dma_start(out=t[:], in_=d[:])
            for c in range(2):
                nc.sync.dma_start(
                    out=W1[:, c * HID : (c + 1) * HID],
                    in_=W1_d[c * P : (c + 1) * P, :],
                )
                nc.sync.dma_start(
                    out=W2[:, c * OUT_DIM : (c + 1) * OUT_DIM],
                    in_=W2_d[c * P : (c + 1) * P, :],
                )
                nc.sync.dma_start(
                    out=m1w[:, c * 128 : (c + 1) * 128],
                    in_=m1w_d[c * P : (c + 1) * P, :],
                )
            # per-layer weight chunk views: chunk c -> [128, HID] AP
            W_chunks = {
                0: [W0[:, :]],
                1: [W1[:, 0:HID], W1[:, HID : 2 * HID]],
                2: [W2[:, 0:OUT_DIM], W2[:, OUT_DIM : 2 * OUT_DIM]],
            }

            # ---- DRAM intermediates ----
            slice1 = dp.tile([NODES_CORE, HID], BF16)
            slice2 = dp.tile([NODES_CORE, HID], BF16)
            h1_sh = dp.tile([NPAD, HID], BF16)
            h2_sh = dp.tile([NPAD, HID], BF16)
            ag_sc = [
                [
                    dp.tile([NCORES * AG_CW, HID], BF16, addr_space="Shared",
                            name=f"agsc{l}_{s}")
                    for s in range(AG_CHUNKS)
                ]
                for l in range(2)
            ]
            pb_in = dp.tile([2 * P + 1, B], F32)
            pb_out = dp.tile([2 * P + 1, B], F32, addr_space="Shared")

            # persistent PSUM for pooled sums (separate banks: matmul
            # start=True resets the whole bank, so groups must not share)
            with tc.tile_pool(name="ppsum", bufs=1, space="PSUM") as ppp:
                ppA = ppp.tile([P, B], F32)
                ppB = ppp.tile([P, B], F32)
                ppC = ppp.tile([P, B], F32)

                def layer(l, tab, idxt, D_in, W, bb, relu, out_slice,
                          ag_fn=None, use_ext=False):
                    Kc = D_in // P  # contraction chunks (1 or 2)
                    with (
                        tc.tile_pool(name=f"hs{l}", bufs=3) as hsp,
                        tc.tile_pool(name=f"sel{l}", bufs=4) as sp,
                        tc.tile_pool(name=f"m{l}", bufs=2) as mp,
                        tc.tile_pool(name=f"mt{l}", bufs=2) as mtp,
                        tc.tile_pool(name=f"h{l}", bufs=2) as hp,
                        tc.tile_pool(name=f"pm{l}", bufs=2, space="PSUM") as pmp,
                        tc.tile_pool(name=f"pt{l}", bufs=1, space="PSUM") as ptp,
                        tc.tile_pool(name=f"ph{l}", bufs=2, space="PSUM") as php,
                        tc.tile_pool(name=f"pool{l}", bufs=2) as polp,
                    ):
                        for w in range(W_CORE):
                            hsb = hsp.tile([P, T_w * D_in], BF16, tag="hs")
                            for t in range(T_w):
                                col = w * T_w + t
                                if use_ext:
                                    rows = (int(ext[t]) + 1) * NCORES * AG_CW
                                    tab_ap = tab[0:rows, :]
                                else:
                                    tab_ap = tab[:]
                                nc.gpsimd.indirect_dma_start(
                                    out=hsb[:, t * D_in : (t + 1) * D_in],
                                    out_offset=None,
                                    in_=tab_ap,
                                    in_offset=bass.IndirectOffsetOnAxis(
                                        ap=idxt[:, col : col + 1], axis=0
                                    ),
                                )
                            pm = pmp.tile([P, D_in], F32, tag="pm")
                            for t in range(T_w):
                                col = w * T_w + t
                                st = sp.tile([P, P], BF16, tag="sel")
                                nc.vector.tensor_scalar(
                                    out=st[:],
                                    in0=iota[:],
                                    scalar1=dstrel[:, col : col + 1],
                                    scalar2=None,
                                    op0=OP.is_equal,
                                )
                                nc.tensor.matmul(
                                    out=pm[:],
                                    lhsT=st[:],
                                    rhs=hsb[:, t * D_in : (t + 1) * D_in],
                                    start=(t == 0),
                                    stop=(t == T_w - 1),
                                )
                            msb = mp.tile([P, D_in], F32, tag="m")
                            nc.vector.tensor_scalar(
                                out=msb[:], in0=pm[:],
                                scalar1=ndst[:, w : w + 1], scalar2=None,
                                op0=OP.mult,
                            )
                            ptt = ptp.tile([P, D_in], F32, tag="pt")
                            for c in range(Kc):
                                nc.tensor.transpose(
                                    out=ptt[:, c * P : (c + 1) * P],
                                    in_=msb[:, c * P : (c + 1) * P],
                                    identity=eye[:],
                                )
                            mtb = mtp.tile([P, D_in], F32, tag="mt")
                            nc.vector.tensor_copy(out=mtb[:], in_=ptt[:])
                            ph = php.tile([P, HID], F32, tag="ph")
                            for c in range(Kc):
                                nc.tensor.matmul(
                                    out=ph[:],
                                    lhsT=mtb[:, c * P : (c + 1) * P],
                                    rhs=W[c],
                                    start=(c == 0),
                                    stop=(c == Kc - 1),
                                )
                            hsb2 = hp.tile([P, HID], F32, tag="h")
                            nc.vector.tensor_tensor(
                                out=hsb2[:], in0=ph[:], in1=bb[:], op=OP.add
                            )
                            if out_slice is not None:
                                # store relu(h)*norm_src as bf16 for the next
                                # layer's gather table (relu(s*x) = s*relu(x))
                                hstore = hp.tile([P, HID], BF16, tag="hst")
                                nc.scalar.activation(
                                    out=hstore[:], in_=hsb2[:], func=ACT.Relu,
                                    scale=nsrc[:, w : w + 1],
                                )
                                nc.sync.dma_start(
                                    out=out_slice[w * P : (w + 1) * P, :],
                                    in_=hstore[:],
                                )
                                if ag_fn is not None and (w + 1) % (W_CORE // AG_CHUNKS) == 0:
                                    ag_fn((w + 1) // (W_CORE // AG_CHUNKS) - 1)
                            else:
                                # ---- pooling contribution (layer 2) ----
                                tmp = polp.tile([P, OUT_DIM], F32, tag="tmp")
                                nc.vector.tensor_tensor(
                                    out=tmp[:], in0=hsb2[:], in1=gw[:], op=OP.mult
                                )
                                gt = polp.tile([P, 1], F32, tag="gt")
                                nc.vector.reduce_sum(
                                    out=gt[:], in_=tmp[:], axis=AX.X
                                )
                                et = polp.tile([P, 1], F32, tag="et")
                                nc.scalar.activation(
                                    out=et[:], in_=gt[:], func=ACT.Exp,
                                    bias=float(gate_b_val), scale=1.0,
                                )
                                he = polp.tile([P, OUT_DIM], F32, tag="he")
                                nc.vector.tensor_scalar_mul(
                                    out=he[:], in0=hsb2[:], scalar1=et[:, :1]
                                )
                                Gt = polp.tile([P, B], F32, tag="G")
                                nc.vector.tensor_scalar(
                                    out=Gt[:],
                                    in0=iota[:, :B],
                                    scalar1=gid[:, w : w + 1],
                                    scalar2=None,
                                    op0=OP.is_equal,
                                )
                                nc.tensor.matmul(
                                    out=ppA[:], lhsT=he[:, 0:P], rhs=Gt[:],
                                    start=(w == 0), stop=(w == W_CORE - 1),
                                    skip_group_check=True,
                                )
                                nc.tensor.matmul(
                                    out=ppB[:], lhsT=he[:, P : 2 * P],
                                    rhs=Gt[:],
                                    start=(w == 0), stop=(w == W_CORE - 1),
                                    skip_group_check=True,
                                )
                                nc.tensor.matmul(
                                    out=ppC[:1, :], lhsT=et[:, :1],
                                    rhs=Gt[:],
                                    start=(w == 0), stop=(w == W_CORE - 1),
                                    skip_group_check=True,
                                )

                def make_ag(sl, hsh, scs):
                    CR = NCORES * AG_CW
                    def ag_fn(s):
                        nc.gpsimd.collective_compute(
                            "AllGather",
                            OP.bypass,
                            replica_groups=[list(range(NCORES))],
                            ins=[sl[s * AG_CW : (s + 1) * AG_CW, :]],
                            outs=[scs[s].opt()],
                        )
                        # chunk-major table: chunk s is contiguous rows
                        nc.sync.dma_start(
                            out=hsh[s * CR : (s + 1) * CR, :],
                            in_=scs[s][:],
                        )
                    return ag_fn

                layer(0, xin, idxs, IN_DIM, W_chunks[0], b0, True, slice1,
                      ag_fn=make_ag(slice1, h1_sh, ag_sc[0]))
                layer(1, h1_sh, idxs2, HID, W_chunks[1], b1, True, slice2,
                      ag_fn=make_ag(slice2, h2_sh, ag_sc[1]), use_ext=True)
                layer(2, h2_sh, idxs2, HID, W_chunks[2], b2, False, None,
                      use_ext=True)

                # ---- pooled partials -> AllReduce ----
                with tc.tile_pool(name="fin", bufs=1) as fp, \
                     tc.tile_pool(name="finp", bufs=1, space="PSUM") as fpp:
                    poolAB = fp.tile([P, 2 * B], F32)
                    poolC = fp.tile([1, B], F32)
                    nc.vector.tensor_copy(out=poolAB[:, 0:B], in_=ppA[:])
                    nc.vector.tensor_copy(out=poolAB[:, B : 2 * B], in_=ppB[:])
                    nc.vector.tensor_copy(out=poolC[:1, :], in_=ppC[:1, :])
                    nc.sync.dma_start(out=pb_in[0:P, :], in_=poolAB[:, 0:B])
                    nc.sync.dma_start(
                        out=pb_in[P : 2 * P, :], in_=poolAB[:, B : 2 * B]
                    )
                    nc.sync.dma_start(
                        out=pb_in[2 * P : 2 * P + 1, :], in_=poolC[:1, :]
                    )
                    nc.gpsimd.collective_compute(
                        "AllReduce",
                        OP.add,
                        replica_groups=[list(range(NCORES))],
                        ins=[pb_in.opt()],
                        outs=[pb_out.opt()],
                    )
                    rAB = fp.tile([P, 2 * B], F32)
                    rC = fp.tile([1, B], F32)
                    nc.sync.dma_start(out=rAB[:, 0:B], in_=pb_out[0:P, :])
                    nc.sync.dma_start(
                        out=rAB[:, B : 2 * B], in_=pb_out[P : 2 * P, :]
                    )
                    nc.sync.dma_start(
                        out=rC[:1, :], in_=pb_out[2 * P : 2 * P + 1, :]
                    )
                    recip = fp.tile([1, B], F32)
                    nc.vector.reciprocal(out=recip[:1, :], in_=rC[:1, :])
                    prr = fpp.tile([P, B], F32, tag="prr")
                    nc.tensor.matmul(
                        out=prr[:], lhsT=ones1[:1, :], rhs=recip[:1, :],
                        start=True, stop=True,
                    )
                    recT = fp.tile([P, B], F32)
                    nc.vector.tensor_copy(out=recT[:], in_=prr[:])
                    pool_s = fp.tile([P, 2 * B], F32)
                    nc.vector.tensor_tensor(
                        out=pool_s[:, 0:B], in0=rAB[:, 0:B], in1=recT[:],
                        op=OP.mult,
                    )
                    nc.vector.tensor_tensor(
                        out=pool_s[:, B : 2 * B], in0=rAB[:, B : 2 * B],
                        in1=recT[:], op=OP.mult,
                    )
                    # ---- MLP ----
                    pz1 = fpp.tile([P, B], F32, tag="pz1")
                    nc.tensor.matmul(
                        out=pz1[:], lhsT=m1w[:, 0:128], rhs=pool_s[:, 0:B],
                        start=True, stop=False,
                    )
                    nc.tensor.matmul(
                        out=pz1[:], lhsT=m1w[:, 128:256],
                        rhs=pool_s[:, B : 2 * B], start=False, stop=True,
                    )
                    z1 = fp.tile([P, B], F32)
                    nc.scalar.activation(
                        out=z1[:], in_=pz1[:], func=ACT.Relu, bias=m1b[:, :1]
                    )
                    pz2 = fpp.tile([64, B], F32, tag="pz2")
                    nc.tensor.matmul(
                        out=pz2[:], lhsT=m2w[:, :], rhs=z1[:],
                        start=True, stop=True,
                    )
                    z2 = fp.tile([64, B], F32)
                    nc.scalar.activation(
                        out=z2[:], in_=pz2[:], func=ACT.Relu, bias=m2b[:, :1]
                    )
                    po = fpp.tile([2, B], F32, tag="po")
                    nc.tensor.matmul(
                        out=po[:], lhsT=m3w[:, :], rhs=z2[:],
                        start=True, stop=True,
                    )
                    ob = fp.tile([2, B], F32)
                    nc.vector.tensor_scalar(
                        out=ob[:2, :], in0=po[:2, :], scalar1=m3b[:2, :1],
                        scalar2=None, op0=OP.add,
                    )
                    nc.sync.dma_start(out=out_d[:, :], in_=ob[:2, :])
                    if debug:
                        nc.sync.dma_start(out=dbg1_d[:], in_=slice1[:])
                        nc.sync.dma_start(out=dbg2_d[:], in_=slice2[:])
                        nc.sync.dma_start(out=dbgp_d[:], in_=pb_out[:])
    return nc


# ---------------------------------------------------------------------------
# Entry point
# ---------------------------------------------------------------------------
def kernel(x, src, dst, graph_ids, W0, b0, W1, b1, W2, b2, gate_w, gate_b,
           m1_w, m1_b, bn1_g, bn1_b, m2_w, m2_b, bn2_g, bn2_b, m3_w, m3_b):
    x = np.asarray(x, np.float32)
    pre = _preprocess(x, np.asarray(src), np.asarray(dst),
                      np.asarray(graph_ids))
    T_w = pre["T_w"]

    s1 = (np.asarray(bn1_g, np.float32) / np.sqrt(np.float32(1.0 + BN_EPS)))
    m1w_f = np.asarray(m1_w, np.float32) * s1[None, :]
    m1b_f = np.asarray(m1_b, np.float32) * s1 + np.asarray(bn1_b, np.float32)
    s2 = (np.asarray(bn2_g, np.float32) / np.sqrt(np.float32(1.0 + BN_EPS)))
    m2w_f = np.asarray(m2_w, np.float32) * s2[None, :]
    m2b_f = np.asarray(m2_b, np.float32) * s2 + np.asarray(bn2_b, np.float32)

    iota = np.broadcast_to(np.arange(P, dtype=np.float32)[None, :], (P, P))
    common = {
        "xin": pre["x_perm"],
        "iota": np.ascontiguousarray(iota),
        "eye": np.eye(P, dtype=np.float32),
        "ones1": np.ones((1, P), np.float32),
        "W0": np.asarray(W0, np.float32),
        "W1": np.asarray(W1, np.float32),
        "W2": np.asarray(W2, np.float32),
        "b0b": np.broadcast_to(np.asarray(b0, np.float32)[None, :], (P, HID)).copy(),
        "b1b": np.broadcast_to(np.asarray(b1, np.float32)[None, :], (P, HID)).copy(),
        "b2b": np.broadcast_to(np.asarray(b2, np.float32)[None, :], (P, OUT_DIM)).copy(),
        "gwb": np.broadcast_to(
            np.asarray(gate_w, np.float32).reshape(1, OUT_DIM), (P, OUT_DIM)
        ).copy(),
        "m1w": m1w_f,
        "m1b": m1b_f.reshape(128, 1),
        "m2w": m2w_f,
        "m2b": m2b_f.reshape(64, 1),
        "m3w": np.asarray(m3_w, np.float32),
        "m3b": np.asarray(m3_b, np.float32).reshape(2, 1),
    }
    in_maps = []
    for c in range(NCORES):
        m = dict(common)
        m["idxs"] = pre["idx_c"][c]
        m["idxs2"] = pre["idx2_c"][c]
        m["nsrc"] = pre["ns_c"][c]
        m["ndst"] = pre["nd_c"][c]
        m["dstrel"] = pre["rel_c"][c]
        m["gid"] = pre["gid_c"][c]
        in_maps.append(m)

    nc = _build_nc(T_w, float(np.asarray(gate_b).reshape(-1)[0]), ext=pre["ext"])
    trace = bool(int(os.environ.get("BASS_GNN_TRACE", "0")))
    res = run_bass_kernel_spmd(nc, in_maps, list(range(NCORES)), trace=trace)
    global LAST_EXEC_NS
    LAST_EXEC_NS = res.exec_time_ns
    out = res.results[0]["out"]  # [2, B]
    return np.ascontiguousarray(out.T.astype(np.float32))  # [B, 2]


LAST_EXEC_NS = None


if __name__ == "__main__":
    # quick self-test against reference if available
    sys.path.insert(0, os.path.dirname(os.path.abspath(__file__)))
    import reference as R

    inputs = {k: np.asarray(v) for k, v in R.setup_inputs().items()}
    got = kernel(**inputs)
    print(got[:4])

